# revision 14
# baseline (speedup 1.0000x reference)
"""BitNet MLP (act_quant -> ternary matmul -> relu^2 -> SubLN -> act_quant ->
ternary matmul) on 8 Trainium2 NeuronCores, data-parallel over tokens.

v2 design (const-g fast path):
- PE does ONLY the 32 mandatory matmuls per 128-token tile (~7.1us);
  both transposes (ix -> xT and iu -> iuT) run on the DMA xbar
  (dma_start_transpose, bf16 SBUF->SBUF, batched 3D out[p,k,c] =
  in[c, 128k+p] which is exactly the per-128-block lhsT layout; verified
  bit-exact on HW).
- ACT (3 big ops/tile): relu drain of the whole 2048-wide ih from one
  contiguous 4-bank PSUM tile; u = Square(dr2 * r) = 127*s/Smax with
  dr2 = sqrt(127/scc) (ACT Sqrt + one Newton step on [P,1]); q2 =
  Square(u) with accum_out = sum(u^2) = dr^2 * sum(s^2).
- DVE: mr = reduce_max(r) (Smax = mr^2); iu = (u + M) - M -> bf16 in one
  2x tensor_scalar; x-quant (xq/ix) at 2x; fused o2 = o_psum * beta
  straight from PSUM; batched beta chain.
- sign(g0) is folded into the final scale (wdk) instead of dr, so dr>0
  and the Square path is valid for negative g0 too.
- Schedule: relu first on ACT and the down-matmul first on PE each
  iteration, so up(t+1) never waits on relu(t); DMA transposes get a
  ~3-iteration completion cushion before their consumer matmuls; beta
  batches of 4 tiles so the PSUM-side o2 scale never waits on the chain.
"""
import os
import numpy as np

import concourse.bass as bass
import concourse.tile as tile
from concourse import mybir
from concourse.bass_utils import run_bass_kernel_spmd

# ---------------------------------------------------------------------------
# Workaround for walrus "Too many sync wait commands" on the TileContext tail
# drain: split the drain's semaphore waits across single-wait SP NOPs, then
# advance the observed clocks so the real drain needs none.
import re as _re
import bass_rust as _bass_rust


def _patched_drain_and_barrier(self, tick_clock, wait_clock):
    gc = tick_clock.global_clock
    ticks = list(map(int, _re.findall(r"\d+", repr(gc))))
    n = len(ticks)
    nonzero = [(i, t) for i, t in enumerate(ticks) if t > 0]
    for i, t in nonzero:
        sub = [0] * n
        sub[i] = t
        sub_scoped = _bass_rust.ScopedClock({None: _bass_rust.VectorClock(sub)})
        nop = self.nc.sync.nop()
        wait_clock.add_sem_waits(nop.ins, sub_scoped)
        for ec in wait_clock.engine_clocks:
            ec.update_past(sub_scoped)
    drain_inst = self.nc.sync.drain()
    wait_clock.add_sem_waits(drain_inst.ins,
                             _bass_rust.ScopedClock({None: gc}))
    self.nc.all_engine_barrier()
    popped = self.nc._tile_sem_poison_stack.pop()
    assert popped is self._sem_poison
    self.nc.clear_and_free_semaphores(list(self.sems.allocated().values()))
    self.nc.all_engine_barrier()


tile.TileContext._drain_and_barrier = _patched_drain_and_barrier


def _split_sync_waits(nc, keep_default=1):
    """walrus caps the number of semaphore waits a single instruction can
    carry (CTRL ops take only 1; compute ops a few). Hoist excess waits onto
    single-wait NOPs inserted immediately before the instruction on the same
    engine — identical semantics, engines execute in order."""
    import dataclasses
    keep_by_op = {}
    proto = None
    for f in nc.m.functions:
        for bb in f.blocks:
            for inst in bb.instructions:
                if type(inst).__name__ == "InstNoOp":
                    proto = inst
                    break
            if proto is not None:
                break
        if proto is not None:
            break
    counter = [0]
    for f in nc.m.functions:
        new_blocks = []
        for bb in f.blocks:
            out = []
            changed = False
            for inst in bb.instructions:
                si = inst.sync_info
                ow = list(si.on_wait) if si is not None and si.on_wait else []
                keep = keep_by_op.get(inst.opcode, keep_default)
                if len(ow) > keep:
                    assert proto is not None, "no NoOp prototype found yet"
                    for w in ow[:-keep]:
                        counter[0] += 1
                        nop = dataclasses.replace(
                            proto,
                            name=f"I-waitsplit-{counter[0]}",
                            engine=inst.engine,
                            sync_info=_bass_rust.SyncInfo(on_wait=[w],
                                                          on_update=[]),
                        )
                        out.append(nop)
                    si.on_wait = ow[-keep:]
                    changed = True
                out.append(inst)
            if changed:
                bb2 = _bass_rust.BasicBlock(name=bb.name, instructions=out)
                bb2.IsExit = bb.IsExit
                bb2.IsLoopEntry = bb.IsLoopEntry
                bb2.IsPredicated = bb.IsPredicated
                new_blocks.append(bb2)
            else:
                new_blocks.append(bb)
        f.blocks = new_blocks
# ---------------------------------------------------------------------------

F32 = mybir.dt.float32
BF16 = mybir.dt.bfloat16
ALU = mybir.AluOpType
AF = mybir.ActivationFunctionType

N_CORES = 8
B, S, H, I = 8, 8192, 512, 2048
TOK = B * S                  # 65536 tokens total
TPC = TOK // N_CORES         # 8192 tokens per core
P = 128                      # partition tile
NT = TPC // P                # 64 token tiles per core
NKH = H // P                 # 4 k-tiles over H
NKI = I // P                 # 16 k-tiles over I
NB = I // 512                # 4 psum banks for the up matmul

MAGIC = 12582912.0           # 1.5 * 2^23: RNE round-to-int trick
EPS = 1e-6                   # SubLN eps (from reference)
BGA = 8                      # tiles per x-stats batch (absmax/scale chain)
BGC = 4                      # tiles per beta batch (keeps o2 lag short)

LAST_RESULT = None           # set by kernel() for test harness introspection


def _emit_weight_quant(nc, tc, consts, warm_ps, wT_dram, n_ktiles, free_len,
                       name, magicb, dve_heavy=False, warm=None):
    """Quantize a (host-pre-transposed) weight matrix to ternary bf16 tiles.

    Streaming two-pass quant with tiny SBUF footprint: the f32 weight data
    is DMA'd TWICE from DRAM through a 2-buffer rotation (pass 1 abs-sums
    each chunk as it lands; pass 2 re-loads each chunk and rounds+clips it
    to ternary).  The extra HBM read (4 MiB) is noise vs the SBUF saved.
    Returns (list of [128, free_len] bf16 tiles, meanclip [128,1]).

    warm: optional callable emitting a HAM-warmup matmul; sprinkled between
    the passes so the PE activity monitor never sees a >3.4us idle window
    during the prologue.
    """
    from contextlib import ExitStack
    n_elem = n_ktiles * 128 * free_len
    half = free_len // 2 if free_len >= 1024 else free_len

    with ExitStack() as ctx:
        stage = ctx.enter_context(tc.tile_pool(name=f"{name}_stage", bufs=2))
        junkp = ctx.enter_context(tc.tile_pool(name=f"{name}_junk", bufs=1))

        # pass 1: per-partition abs sums, chunk-streamed.  dve_heavy puts
        # everything on DVE; otherwise alternate ACT/DVE by k.
        asum = consts.tile([P, n_ktiles], F32, tag=f"{name}_asum")
        junk = junkp.tile([P, free_len], BF16, tag="junk")
        for k in range(n_ktiles):
            wf = stage.tile([P, free_len], F32, tag="wf", bufs=2)
            nc.sync.dma_start(out=wf, in_=wT_dram[k * P:(k + 1) * P, :])
            if dve_heavy or (k % 2 == 1):
                nc.vector.tensor_reduce(out=asum[:, k:k + 1],
                                        in_=wf,
                                        axis=mybir.AxisListType.X,
                                        op=ALU.add,
                                        apply_absolute_value=True)
            else:
                nc.scalar.activation(out=junk, in_=wf, func=AF.Abs,
                                     accum_out=asum[:, k:k + 1])
            if warm is not None and k % 4 == 1:
                warm()
        tot = consts.tile([P, 1], F32, tag=f"{name}_tot")
        nc.vector.tensor_reduce(out=tot, in_=asum, axis=mybir.AxisListType.X,
                                op=ALU.add)
        # broadcast-sum across partitions: ones128.T @ tot into the warm-up
        # PSUM bank (prologue-time; ring cycles are free)
        ones128 = junkp.tile([P, P], F32, tag="ones128")
        nc.vector.memset(ones128, 1.0)
        nc.tensor.matmul(out=warm_ps[:, 0:1], lhsT=ones128, rhs=tot,
                         start=True, stop=True)
        gsum = consts.tile([P, 1], F32, tag=f"{name}_gsum")
        nc.scalar.copy(out=gsum, in_=warm_ps[:, 0:1])
        # mean -> clip -> reciprocal scale
        meanclip = consts.tile([P, 1], F32, tag=f"{name}_meanclip")
        nc.vector.tensor_scalar(out=meanclip, in0=gsum, scalar1=1.0 / n_elem,
                                scalar2=1e-5, op0=ALU.mult, op1=ALU.max)
        swq = consts.tile([P, 1], F32, tag=f"{name}_swq")
        nc.vector.reciprocal(out=swq, in_=meanclip)

        # pass 2: re-DMA each chunk, round+clip to ternary bf16 in
        # half-chunks (keeps the rt/cl scratch tiles small)
        wq_tiles = []
        for k in range(n_ktiles):
            wf = stage.tile([P, free_len], F32, tag="wf", bufs=2)
            nc.sync.dma_start(out=wf, in_=wT_dram[k * P:(k + 1) * P, :])
            wq = consts.tile([P, free_len], BF16, tag=f"{name}_wq{k}")
            for h0 in range(0, free_len, half):
                hs = slice(h0, h0 + half)
                rt = junkp.tile([P, half], F32, tag="stage_rt", bufs=1)
                if dve_heavy or (k % 2 == 1):
                    nc.vector.tensor_scalar(out=rt, in0=wf[:, hs],
                                            scalar1=swq, scalar2=MAGIC,
                                            op0=ALU.mult, op1=ALU.add)
                else:
                    nc.scalar.activation(out=rt, in_=wf[:, hs],
                                         func=AF.Identity, bias=magicb,
                                         scale=swq)
                cl = junkp.tile([P, half], F32, tag="stage_cl", bufs=1)
                nc.vector.tensor_scalar(out=cl, in0=rt, scalar1=MAGIC,
                                        scalar2=1.0, op0=ALU.subtract,
                                        op1=ALU.min)
                nc.vector.tensor_scalar(out=wq[:, hs], in0=cl, scalar1=-1.0,
                                        scalar2=None, op0=ALU.max)
            wq_tiles.append(wq)
            if warm is not None and k % 4 == 3:
                warm()
    return wq_tiles, meanclip


def build_nc(general_g: bool):
    nc = bass.Bass()
    x_d = nc.dram_tensor("x", [TPC, H], F32, kind="ExternalInput")
    wupT_d = nc.dram_tensor("wupT", [H, I], F32, kind="ExternalInput")
    wdnT_d = nc.dram_tensor("wdnT", [I, H], F32, kind="ExternalInput")
    g_d = nc.dram_tensor("g", [I], F32, kind="ExternalInput")
    out_d = nc.dram_tensor("out", [TPC, H], F32, kind="ExternalOutput")

    from contextlib import ExitStack
    with ExitStack() as ctx:
        tc = ctx.enter_context(tile.TileContext(nc))

        # ---------------- constants / weight prep ----------------
        consts = ctx.enter_context(tc.tile_pool(name="consts", bufs=1))

        magicb = consts.tile([P, 1], F32)
        nc.vector.memset(magicb, MAGIC)
        nmagicb = consts.tile([P, 1], F32)
        nc.vector.memset(nmagicb, -MAGIC)

        g_bc = None
        if general_g:
            # g broadcast to all partitions: [128, I] f32
            g_bc = consts.tile([P, I], F32)
            g_ap = g_d[:]
            g_bcast_ap = bass.AP(tensor=g_ap.tensor, offset=g_ap.offset,
                                 ap=[[0, P]] + list(g_ap.ap))
            nc.gpsimd.dma_start(out=g_bc, in_=g_bcast_ap)

        # ---------------- PSUM pools (8 banks total) ----------------
        # ih: one contiguous 4-bank tile [P, 4, 512] f32; o: 4 banks.
        # The prologue (warm-up matmuls, weight-quant broadcast, g0
        # broadcast) borrows one o slot before the first down-matmul.
        ps_ih = ctx.enter_context(tc.tile_pool(name="ps_ih", bufs=1,
                                               space="PSUM"))
        ps_o = ctx.enter_context(tc.tile_pool(name="ps_o", bufs=4,
                                              space="PSUM"))

        # HAM warm-up: tiny dependency-free matmuls sprinkled through the
        # prologue emission keep the PE activity monitor at K=8/8 so the
        # first real matmuls run at 2.4 GHz instead of 1.2 GHz.
        warm_w = consts.tile([P, P], BF16)
        nc.vector.memset(warm_w, 1.0)
        warm_ps = ps_o.tile([P, H], F32, tag="o", name="warm_ps")

        def warm():
            nc.tensor.matmul(out=warm_ps[:, 0:P], lhsT=warm_w,
                             rhs=warm_w, start=True, stop=True)

        for _ in range(6):
            warm()

        g0b = consts.tile([P, 1], F32)
        with ExitStack() as gctx:
            gstage = gctx.enter_context(tc.tile_pool(name="gstage", bufs=1))
            # g0 broadcast [128,1] via K=1 matmul with ones
            ones_row = gstage.tile([1, P], F32, tag="ones_row")
            nc.vector.memset(ones_row, 1.0)
            g0_sb = gstage.tile([1, 1], F32, tag="g0sb")
            nc.gpsimd.dma_start(out=g0_sb, in_=g_d[0:1])
            nc.tensor.matmul(out=warm_ps[:, 0:1], lhsT=ones_row, rhs=g0_sb,
                             start=True, stop=True)
            nc.scalar.copy(out=g0b, in_=warm_ps[:, 0:1])

        # weights are quantized mid-prologue (below); placeholders for the
        # emit closures, assigned before first use.
        wup_q = wdn_q = None
        k1b = consts.tile([P, 1], F32)
        wdk = consts.tile([P, 1], F32)
        isg = consts.tile([P, 1], F32)
        g0a = consts.tile([P, 1], F32)

        def emit_gain_consts(up_meanclip, dn_meanclip):
            nc.vector.tensor_scalar_mul(out=k1b, in0=up_meanclip,
                                        scalar1=1.0 / 127.0)
            nc.scalar.activation(out=g0a, in_=g0b, func=AF.Abs)
            if general_g:
                # per-channel sign lives in sg/iu; wdk unsigned
                nc.vector.tensor_scalar_mul(out=wdk, in0=dn_meanclip,
                                            scalar1=1.0 / 127.0)
                nc.vector.memset(isg, 1.0 / 127.0)
            else:
                # dr is computed positive (Square path); fold sign(g0)
                # into the final output scale instead.
                sgn = consts.tile([P, 1], F32)
                nc.scalar.activation(out=sgn, in_=g0b, func=AF.Sign)
                wdk0 = consts.tile([P, 1], F32)
                nc.vector.tensor_scalar_mul(out=wdk0, in0=dn_meanclip,
                                            scalar1=1.0 / 127.0)
                nc.vector.tensor_tensor(out=wdk, in0=wdk0, in1=sgn,
                                        op=ALU.mult)

        # ---------------- main token-tile pipeline ----------------
        xs_bufs = 9 if general_g else 12
        xs_pool = ctx.enter_context(tc.tile_pool(name="xs", bufs=xs_bufs))
        xq_pool = ctx.enter_context(tc.tile_pool(name="xqp", bufs=1))
        ix_pool = ctx.enter_context(tc.tile_pool(name="ixp", bufs=2))
        xT_pool = ctx.enter_context(tc.tile_pool(name="xTp", bufs=5))
        rp = ctx.enter_context(
            tc.tile_pool(name="rp", bufs=(2 if general_g else 3)))
        up = ctx.enter_context(tc.tile_pool(name="up", bufs=2))
        sp = (ctx.enter_context(tc.tile_pool(name="sp", bufs=3))
              if general_g else None)
        sgp = (ctx.enter_context(tc.tile_pool(name="sgp", bufs=2))
               if general_g else None)
        iup = ctx.enter_context(tc.tile_pool(name="iup", bufs=2))
        iuTp = ctx.enter_context(tc.tile_pool(name="iuTp", bufs=4))
        junkp = ctx.enter_context(tc.tile_pool(name="mjunk", bufs=1))
        o2p = ctx.enter_context(tc.tile_pool(name="o2p", bufs=2))
        small = ctx.enter_context(tc.tile_pool(name="small", bufs=3))
        batchp = ctx.enter_context(tc.tile_pool(name="batchp", bufs=3))

        KV = (1.0 / I) if general_g else (1.0 / (127.0 * 127.0 * I))

        abatch = {}        # ib8 -> x batch state (x_tiles, xm8, t08, xsc8)
        cbatch = {}        # ib4 -> beta batch state (Sm4, q24)
        tile_state = {}    # (key, t) -> live tile
        c_state = {}       # ib4 -> b4 output-scale tile

        def emit_Xdma(t):
            """One x-tile load per iteration (sync ring, emitted last)."""
            ib = (t // BGA) * BGA
            if t == ib:
                abatch[ib] = dict(x_tiles={}, xm8=batchp.tile(
                    [P, BGA], F32, tag="xm8", name="xm8"))
            x_sb = xs_pool.tile([P, H], F32, tag="x", bufs=xs_bufs)
            nc.sync.dma_start(out=x_sb, in_=x_d[t * P:(t + 1) * P, :])
            abatch[ib]["x_tiles"][t] = x_sb

        def emit_XM(t):
            """One per-token absmax per iteration."""
            ib = (t // BGA) * BGA
            bs = abatch[ib]
            j = t - ib
            nc.vector.tensor_reduce(out=bs["xm8"][:, j:j + 1],
                                    in_=bs["x_tiles"][t],
                                    axis=mybir.AxisListType.X, op=ALU.max,
                                    apply_absolute_value=True)

        def emit_XC(ib):
            """Batched x-scale chain for batch ib (after all 8 absmaxes)."""
            bs = abatch[ib]
            t08 = batchp.tile([P, BGA], F32, tag="t08")
            nc.vector.tensor_scalar_max(out=t08, in0=bs["xm8"], scalar1=1e-5)
            xr8 = batchp.tile([P, BGA], F32, tag="xr8")
            nc.vector.reciprocal(out=xr8, in_=t08)
            xsc8 = batchp.tile([P, BGA], F32, tag="xsc8")
            nc.vector.tensor_scalar_mul(out=xsc8, in0=xr8, scalar1=127.0)
            bs["t08"] = t08
            bs["xsc8"] = xsc8

        def emit_Q(t):
            """x-quant for tile t (DVE 2x) + xbar DMA transpose to xT."""
            ib = (t // BGA) * BGA
            bs = abatch[ib]
            j = t - ib
            x_sb = bs["x_tiles"].pop(t)
            xq = xq_pool.tile([P, H], F32, tag="xq")
            nc.vector.tensor_scalar(out=xq, in0=x_sb,
                                    scalar1=bs["xsc8"][:, j:j + 1],
                                    scalar2=MAGIC, op0=ALU.mult, op1=ALU.add)
            ix = ix_pool.tile([P, H], BF16, tag="ix")
            nc.vector.tensor_scalar(out=ix, in0=xq, scalar1=MAGIC,
                                    scalar2=None, op0=ALU.subtract)
            xT = xT_pool.tile([P, NKH, P], BF16, tag="xT")
            nc.sync.dma_start_transpose(out=xT, in_=ix)
            tile_state[("xT", t)] = xT

        def emit_U(t):
            """Up matmul for tile t into the single 4-bank ih PSUM tile."""
            xT = tile_state.pop(("xT", t))
            ih = ps_ih.tile([P, NB, 512], F32, tag="ih")
            for q in range(NB):
                for k in range(NKH):
                    nc.tensor.matmul(
                        out=ih[:, q, :],
                        lhsT=xT[:, k, :],
                        rhs=wup_q[k][:, q * 512:(q + 1) * 512],
                        start=(k == 0), stop=(k == NKH - 1))
            tile_state[("ih", t)] = ih

        def emit_R(t):
            """relu drain of the whole 2048-wide ih in ONE ACT op."""
            ih = tile_state.pop(("ih", t))
            r_sb = rp.tile([P, I], F32, tag="r")
            nc.scalar.activation(out=r_sb,
                                 in_=ih.rearrange("p a b -> p (a b)"),
                                 func=AF.Relu)
            tile_state[("r", t)] = r_sb

        def emit_S(t):
            """Per-token max + quant-scale chain.  const-g: mr = max(r),
            scc = max(mr^2, 1e-30), a = 127/scc, y0 = ACT Sqrt(a)
            (refined to dr2 in emit_S_post).  general-g: s = r*r,
            sg = s*g, Smax = max|sg|, dr = 127/clip(Smax)."""
            ib = (t // BGC) * BGC
            j = t - ib
            if t == ib:
                cbatch[ib] = dict(
                    Sm4=batchp.tile([P, BGC], F32, tag="Sm4", name="Sm4"),
                    q24=batchp.tile([P, BGC], F32, tag="q24", name="q24"))
            cs = cbatch[ib]
            r_sb = tile_state[("r", t)]
            if general_g:
                s_sb = sp.tile([P, I], F32, tag="s")
                nc.vector.tensor_tensor(out=s_sb, in0=r_sb, in1=r_sb,
                                        op=ALU.mult)
                sg = sgp.tile([P, I], F32, tag="sg")
                nc.vector.tensor_tensor(out=sg, in0=s_sb, in1=g_bc,
                                        op=ALU.mult)
                nc.vector.tensor_reduce(out=cs["Sm4"][:, j:j + 1], in_=sg,
                                        axis=mybir.AxisListType.X, op=ALU.max,
                                        apply_absolute_value=True)
                sc2 = small.tile([P, 1], F32, tag="sc2")
                nc.vector.tensor_scalar(out=sc2, in0=cs["Sm4"][:, j:j + 1],
                                        scalar1=1e-30, scalar2=isg,
                                        op0=ALU.max, op1=ALU.mult)
                dr = small.tile([P, 1], F32, tag="dr")
                nc.vector.reciprocal(out=dr, in_=sc2)
                del tile_state[("r", t)]
                tile_state[("s", t)] = s_sb
                tile_state[("sg", t)] = sg
                tile_state[("dr2", t)] = dr
                return
            mr = small.tile([P, 1], F32, tag="mr")
            nc.vector.tensor_reduce(out=mr, in_=r_sb,
                                    axis=mybir.AxisListType.X, op=ALU.max)
            # scc = max(mr^2, 1e-30) (== max of f32-rounded squares)
            nc.vector.tensor_scalar(out=cs["Sm4"][:, j:j + 1], in0=mr,
                                    scalar1=mr, scalar2=1e-30,
                                    op0=ALU.mult, op1=ALU.max)
            s1 = small.tile([P, 1], F32, tag="s1")
            nc.vector.tensor_scalar_mul(out=s1, in0=cs["Sm4"][:, j:j + 1],
                                        scalar1=1.0 / 127.0)
            adr = small.tile([P, 1], F32, tag="adr")
            nc.vector.reciprocal(out=adr, in_=s1)          # a = 127/scc
            y0 = small.tile([P, 1], F32, tag="y0")
            nc.scalar.activation(out=y0, in_=adr, func=AF.Sqrt)
            tile_state[("adr", t)] = adr
            tile_state[("y0", t)] = y0

        def emit_S_post(t):
            """Newton step: dr2 = 0.5*(y0 + a/y0) ~ sqrt(a) to ~1e-7."""
            if general_g:
                return
            adr = tile_state.pop(("adr", t))
            y0 = tile_state.pop(("y0", t))
            r0 = small.tile([P, 1], F32, tag="r0")
            nc.vector.reciprocal(out=r0, in_=y0)
            qn = small.tile([P, 1], F32, tag="qn")
            nc.vector.tensor_tensor(out=qn, in0=adr, in1=r0, op=ALU.mult)
            wn = small.tile([P, 1], F32, tag="wn")
            nc.vector.tensor_tensor(out=wn, in0=y0, in1=qn, op=ALU.add)
            dr2 = small.tile([P, 1], F32, tag="dr2")
            nc.vector.tensor_scalar_mul(out=dr2, in0=wn, scalar1=0.5)
            tile_state[("dr2", t)] = dr2

        def emit_T(t):
            """const: u = Square(dr2 * r) = 127*s/scc (exact quant values
            pre-round).  general: u = dr*sg + MAGIC via Identity fma."""
            dr2 = tile_state.pop(("dr2", t))
            u = up.tile([P, I], F32, tag="u")
            if general_g:
                sg = tile_state.pop(("sg", t))
                nc.scalar.activation(out=u, in_=sg, func=AF.Identity,
                                     bias=magicb, scale=dr2)
            else:
                r_sb = tile_state.pop(("r", t))
                nc.scalar.activation(out=u, in_=r_sb, func=AF.Square,
                                     scale=dr2)
            tile_state[("u", t)] = u

        def emit_I(t):
            """iu -> bf16 (one 2x DVE op) + xbar DMA transpose to iuT."""
            u = tile_state.pop(("u", t))
            iu = iup.tile([P, I], BF16, tag="iu")
            if general_g:
                # u already == dr*sg + MAGIC
                nc.vector.tensor_scalar(out=iu, in0=u, scalar1=MAGIC,
                                        scalar2=None, op0=ALU.subtract)
            else:
                nc.vector.tensor_scalar(out=iu, in0=u, scalar1=MAGIC,
                                        scalar2=MAGIC, op0=ALU.add,
                                        op1=ALU.subtract)
            iuT = iuTp.tile([P, NKI, P], BF16, tag="iuT")
            nc.sync.dma_start_transpose(out=iuT, in_=iu)
            tile_state[("iuT", t)] = iuT

        def emit_Q2(t):
            """q2 accumulation on ACT.  const: q24 col = sum(u^2) =
            dr^2*sum(s^2).  general: raw sum(s^2) (KV = 1/I)."""
            ib = (t // BGC) * BGC
            j = t - ib
            cs = cbatch[ib]
            junk2 = junkp.tile([P, I], BF16, tag="junk2")
            if general_g:
                s_sb = tile_state.pop(("s", t))
                nc.scalar.activation(out=junk2, in_=s_sb, func=AF.Square,
                                     accum_out=cs["q24"][:, j:j + 1])
            else:
                u = tile_state[("u", t)]
                nc.scalar.activation(out=junk2, in_=u, func=AF.Square,
                                     accum_out=cs["q24"][:, j:j + 1])

        def emit_D(t):
            """Down matmul for tile t (lhsT = DMA-transposed iuT blocks)."""
            iuT = tile_state.pop(("iuT", t))
            o_ps = ps_o.tile([P, H], F32, tag="o")
            for k in range(NKI):
                nc.tensor.matmul(out=o_ps,
                                 lhsT=iuT[:, k, :],
                                 rhs=wdn_q[k],
                                 start=(k == 0), stop=(k == NKI - 1))
            tile_state[("ops", t)] = o_ps

        def emit_C(ib):
            """Batched beta chain for tiles ib..ib+BGC-1."""
            cs = cbatch.pop(ib)
            iba = (ib // BGA) * BGA
            bs = abatch[iba]
            j4 = ib - iba
            t04 = bs["t08"][:, j4:j4 + BGC]
            Sm4, q24 = cs["Sm4"], cs["q24"]
            if general_g:
                scc4 = batchp.tile([P, BGC], F32, tag="scc4")
                nc.vector.tensor_scalar_max(out=scc4, in0=Sm4, scalar1=1e-30)
            else:
                scc4 = Sm4  # already max(mr^2, 1e-30) from emit_S
            ga4 = batchp.tile([P, BGC], F32, tag="ga4")
            nc.vector.tensor_scalar_mul(out=ga4, in0=t04, scalar1=k1b)
            al4 = batchp.tile([P, BGC], F32, tag="al4")
            nc.vector.tensor_tensor(out=al4, in0=ga4, in1=ga4, op=ALU.mult)
            m14 = batchp.tile([P, BGC], F32, tag="m14")
            nc.vector.tensor_tensor(out=m14, in0=al4, in1=scc4, op=ALU.mult)
            v14 = batchp.tile([P, BGC], F32, tag="v14")
            al24 = batchp.tile([P, BGC], F32, tag="al24")
            nc.vector.tensor_tensor(out=al24, in0=al4, in1=al4, op=ALU.mult)
            if general_g:
                nc.vector.tensor_tensor(out=v14, in0=al24, in1=q24,
                                        op=ALU.mult)
            else:
                ss4 = batchp.tile([P, BGC], F32, tag="ss4")
                nc.vector.tensor_tensor(out=ss4, in0=scc4, in1=scc4,
                                        op=ALU.mult)
                qs4 = batchp.tile([P, BGC], F32, tag="qs4")
                nc.vector.tensor_tensor(out=qs4, in0=q24, in1=ss4,
                                        op=ALU.mult)
                nc.vector.tensor_tensor(out=v14, in0=al24, in1=qs4,
                                        op=ALU.mult)
            Ve4 = batchp.tile([P, BGC], F32, tag="Ve4")
            nc.vector.tensor_scalar(out=Ve4, in0=v14, scalar1=KV,
                                    scalar2=EPS, op0=ALU.mult, op1=ALU.add)
            sq4 = batchp.tile([P, BGC], F32, tag="sq4")
            nc.scalar.activation(out=sq4, in_=Ve4, func=AF.Sqrt)
            cr4 = batchp.tile([P, BGC], F32, tag="cr4")
            nc.vector.reciprocal(out=cr4, in_=sq4)
            # one Newton step for rsqrt accuracy (ACT sqrt is approximate)
            h14 = batchp.tile([P, BGC], F32, tag="h14")
            nc.vector.tensor_tensor(out=h14, in0=cr4, in1=cr4, op=ALU.mult)
            h24 = batchp.tile([P, BGC], F32, tag="h24")
            nc.vector.tensor_tensor(out=h24, in0=h14, in1=Ve4, op=ALU.mult)
            h34 = batchp.tile([P, BGC], F32, tag="h34")
            nc.vector.tensor_scalar(out=h34, in0=h24, scalar1=-0.5,
                                    scalar2=1.5, op0=ALU.mult, op1=ALU.add)
            c4 = batchp.tile([P, BGC], F32, tag="c4")
            nc.vector.tensor_tensor(out=c4, in0=cr4, in1=h34, op=ALU.mult)
            if general_g:
                m1g4 = m14
            else:
                m1g4 = batchp.tile([P, BGC], F32, tag="m1g4")
                nc.vector.tensor_scalar_mul(out=m1g4, in0=m14, scalar1=g0a)
            mu4 = batchp.tile([P, BGC], F32, tag="mu4")
            nc.vector.tensor_tensor(out=mu4, in0=c4, in1=m1g4, op=ALU.mult)
            b4 = batchp.tile([P, BGC], F32, tag="b4")
            nc.vector.tensor_scalar(out=b4, in0=mu4, scalar1=1e-5,
                                    scalar2=wdk, op0=ALU.max, op1=ALU.mult)
            c_state[ib] = b4
            if ib % BGA == BGA - BGC:
                del abatch[iba]

        def emit_O(t):
            """Fused scale+drain of the down PSUM on DVE, then store."""
            ib = (t // BGC) * BGC
            j = t - ib
            b4 = c_state[ib]
            o_ps = tile_state.pop(("ops", t))
            o2 = o2p.tile([P, H], F32, tag="o2")
            nc.vector.tensor_scalar_mul(out=o2, in0=o_ps,
                                        scalar1=b4[:, j:j + 1])
            nc.sync.dma_start(out=out_d[t * P:(t + 1) * P, :], in_=o2)
            if j == BGC - 1:
                del c_state[ib]

        # ---- software-pipelined emission ----
        # Stage lags (iteration i):
        #   R(i) | T(i-2) | S(i-1) | Q2(i-3) | C(i-8 batch) | O(i-8) |
        #   I(i-3) | Q(i+4) | D(i-6) | U(i+1) | S_post(i-1) | XM(i+12) |
        #   XC(i+5 batch) | Xdma(i+13)
        # Per-engine program order per iteration:
        #   ACT: R, u(T), Sqrt(S), q2(Q2)       (relu first: U(i+1) waits)
        #   DVE: S-pre, C, O.o2, I.iu, Q.xq/ix, S_post, XM, XC
        #   PE : D, U                           (down first: relu cushion)
        #   SYNC: O.out-dma, I.iuT-dma, Q.xT-dma, Xdma     (x load last)
        wup_q, up_meanclip = _emit_weight_quant(
            nc, tc, consts, warm_ps, wupT_d, NKH, I, "wup", magicb,
            warm=warm)
        for t in range(13):
            emit_Xdma(t)
        for t in range(12):
            emit_XM(t)
        emit_XC(0)
        warm()
        emit_Q(0)
        emit_Q(1)
        warm()
        emit_Q(2)
        emit_Q(3)
        warm()
        emit_U(0)
        wdn_q, dn_meanclip = _emit_weight_quant(
            nc, tc, consts, warm_ps, wdnT_d, NKI, H, "wdn", magicb,
            dve_heavy=True, warm=None)
        emit_gain_consts(up_meanclip, dn_meanclip)
        for i in range(NT + 13):
            if 0 <= i < NT:
                emit_R(i)
            if 0 <= i - 2 < NT:
                emit_T(i - 2)
            if 0 <= i - 1 < NT:
                emit_S(i - 1)
            if 0 <= i - 3 < NT:
                emit_Q2(i - 3)
            if (i - 8) % BGC == 0 and 0 <= i - 8 < NT:
                emit_C(i - 8)
            if 0 <= i - 8 < NT:
                emit_O(i - 8)
            if 0 <= i - 3 < NT:
                emit_I(i - 3)
            if 0 <= i + 4 < NT:
                emit_Q(i + 4)
            if 0 <= i - 6 < NT:
                emit_D(i - 6)
            if 0 <= i + 1 < NT:
                emit_U(i + 1)
            if 0 <= i - 1 < NT:
                emit_S_post(i - 1)
            if 0 <= i + 12 < NT:
                emit_XM(i + 12)
            if (i + 5) % BGA == 0 and 0 <= i + 5 < NT:
                emit_XC(i + 5)
            if 0 <= i + 13 < NT:
                emit_Xdma(i + 13)

    _split_sync_waits(nc)
    return nc


_NC_CACHE = {}


def kernel(x, w_up, w_down, g):
    global LAST_RESULT
    x = np.ascontiguousarray(x, dtype=np.float32)
    w_up = np.ascontiguousarray(w_up, dtype=np.float32)
    w_down = np.ascontiguousarray(w_down, dtype=np.float32)
    g = np.ascontiguousarray(g, dtype=np.float32)

    if abs(float(g[0])) < 1e-30 and np.all(g == g[0]):
        return np.zeros_like(x)

    general = not bool(np.all(g == g[0]))
    key = ("gen" if general else "const")
    if key not in _NC_CACHE:
        _NC_CACHE[key] = build_nc(general)
    nc = _NC_CACHE[key]

    xt = x.reshape(TOK, H)
    wupT = np.ascontiguousarray(w_up.T)    # [H, I]
    wdnT = np.ascontiguousarray(w_down.T)  # [I, H]
    in_maps = [
        {"x": xt[c * TPC:(c + 1) * TPC], "wupT": wupT, "wdnT": wdnT, "g": g}
        for c in range(N_CORES)
    ]
    res = run_bass_kernel_spmd(
        nc, in_maps, list(range(N_CORES)),
        trace=bool(os.environ.get("BASS_TRACE")),
    )
    LAST_RESULT = res
    out = np.concatenate([res.results[c]["out"] for c in range(N_CORES)],
                         axis=0)
    return out.reshape(B, S, H)


# revision 19
# speedup vs baseline: 1.0295x; 1.0295x over previous
"""BitNet MLP (act_quant -> ternary matmul -> relu^2 -> SubLN -> act_quant ->
ternary matmul) on 8 Trainium2 NeuronCores, data-parallel over tokens.

v2 design (const-g fast path):
- PE does ONLY the 32 mandatory matmuls per 128-token tile (~7.1us);
  both transposes (ix -> xT and iu -> iuT) run on the DMA xbar
  (dma_start_transpose, bf16 SBUF->SBUF, batched 3D out[p,k,c] =
  in[c, 128k+p] which is exactly the per-128-block lhsT layout; verified
  bit-exact on HW).
- ACT (3 big ops/tile): relu drain of the whole 2048-wide ih from one
  contiguous 4-bank PSUM tile; u = Square(dr2 * r) = 127*s/Smax with
  dr2 = sqrt(127/scc) (ACT Sqrt + one Newton step on [P,1]); q2 =
  Square(u) with accum_out = sum(u^2) = dr^2 * sum(s^2).
- DVE: mr = reduce_max(r) (Smax = mr^2); iu = (u + M) - M -> bf16 in one
  2x tensor_scalar; x-quant (xq/ix) at 2x; fused o2 = o_psum * beta
  straight from PSUM; batched beta chain.
- sign(g0) is folded into the final scale (wdk) instead of dr, so dr>0
  and the Square path is valid for negative g0 too.
- Schedule: relu first on ACT and the down-matmul first on PE each
  iteration, so up(t+1) never waits on relu(t); DMA transposes get a
  ~3-iteration completion cushion before their consumer matmuls; beta
  batches of 4 tiles so the PSUM-side o2 scale never waits on the chain.
"""
import os
import numpy as np

import concourse.bass as bass
import concourse.tile as tile
from concourse import mybir
from concourse.bass_utils import run_bass_kernel_spmd

# ---------------------------------------------------------------------------
# Workaround for walrus "Too many sync wait commands" on the TileContext tail
# drain: split the drain's semaphore waits across single-wait SP NOPs, then
# advance the observed clocks so the real drain needs none.
import re as _re
import bass_rust as _bass_rust


def _patched_drain_and_barrier(self, tick_clock, wait_clock):
    gc = tick_clock.global_clock
    ticks = list(map(int, _re.findall(r"\d+", repr(gc))))
    n = len(ticks)
    nonzero = [(i, t) for i, t in enumerate(ticks) if t > 0]
    for i, t in nonzero:
        sub = [0] * n
        sub[i] = t
        sub_scoped = _bass_rust.ScopedClock({None: _bass_rust.VectorClock(sub)})
        nop = self.nc.sync.nop()
        wait_clock.add_sem_waits(nop.ins, sub_scoped)
        for ec in wait_clock.engine_clocks:
            ec.update_past(sub_scoped)
    drain_inst = self.nc.sync.drain()
    wait_clock.add_sem_waits(drain_inst.ins,
                             _bass_rust.ScopedClock({None: gc}))
    self.nc.all_engine_barrier()
    popped = self.nc._tile_sem_poison_stack.pop()
    assert popped is self._sem_poison
    self.nc.clear_and_free_semaphores(list(self.sems.allocated().values()))
    self.nc.all_engine_barrier()


tile.TileContext._drain_and_barrier = _patched_drain_and_barrier


def _split_sync_waits(nc, keep_default=1):
    """walrus caps the number of semaphore waits a single instruction can
    carry (CTRL ops take only 1; compute ops a few). Hoist excess waits onto
    single-wait NOPs inserted immediately before the instruction on the same
    engine — identical semantics, engines execute in order."""
    import dataclasses
    keep_by_op = {}
    proto = None
    for f in nc.m.functions:
        for bb in f.blocks:
            for inst in bb.instructions:
                if type(inst).__name__ == "InstNoOp":
                    proto = inst
                    break
            if proto is not None:
                break
        if proto is not None:
            break
    counter = [0]
    for f in nc.m.functions:
        new_blocks = []
        for bb in f.blocks:
            out = []
            changed = False
            for inst in bb.instructions:
                si = inst.sync_info
                ow = list(si.on_wait) if si is not None and si.on_wait else []
                keep = keep_by_op.get(inst.opcode, keep_default)
                if len(ow) > keep:
                    assert proto is not None, "no NoOp prototype found yet"
                    for w in ow[:-keep]:
                        counter[0] += 1
                        nop = dataclasses.replace(
                            proto,
                            name=f"I-waitsplit-{counter[0]}",
                            engine=inst.engine,
                            sync_info=_bass_rust.SyncInfo(on_wait=[w],
                                                          on_update=[]),
                        )
                        out.append(nop)
                    si.on_wait = ow[-keep:]
                    changed = True
                out.append(inst)
            if changed:
                bb2 = _bass_rust.BasicBlock(name=bb.name, instructions=out)
                bb2.IsExit = bb.IsExit
                bb2.IsLoopEntry = bb.IsLoopEntry
                bb2.IsPredicated = bb.IsPredicated
                new_blocks.append(bb2)
            else:
                new_blocks.append(bb)
        f.blocks = new_blocks
# ---------------------------------------------------------------------------

F32 = mybir.dt.float32
BF16 = mybir.dt.bfloat16
ALU = mybir.AluOpType
AF = mybir.ActivationFunctionType

N_CORES = 8
B, S, H, I = 8, 8192, 512, 2048
TOK = B * S                  # 65536 tokens total
TPC = TOK // N_CORES         # 8192 tokens per core
P = 128                      # partition tile
NT = TPC // P                # 64 token tiles per core
NKH = H // P                 # 4 k-tiles over H
NKI = I // P                 # 16 k-tiles over I
NB = I // 512                # 4 psum banks for the up matmul

MAGIC = 12582912.0           # 1.5 * 2^23: RNE round-to-int trick
EPS = 1e-6                   # SubLN eps (from reference)
BGA = 8                      # tiles per x-stats batch (absmax/scale chain)
BGC = 4                      # tiles per beta batch (keeps o2 lag short)

LAST_RESULT = None           # set by kernel() for test harness introspection


def _emit_weight_quant(nc, tc, consts, warm_ps, wT_dram, n_ktiles, free_len,
                       name, magicb, dve_heavy=False, warm=None,
                       quarter=False):
    """Quantize a (host-pre-transposed) weight matrix to ternary bf16 tiles.

    Fully staged: one DMA per chunk into scoped SBUF f32 tiles (no WAR
    stalls on the sync ring), then two passes over SBUF (abs-sum, then
    round+clip in half-chunks).  Returns (list of [128, free_len] bf16
    tiles, meanclip [128,1]).

    warm: optional callable emitting a HAM-warmup matmul; sprinkled between
    the passes so the PE activity monitor never sees a >3.4us idle window
    during the prologue.
    """
    from contextlib import ExitStack
    n_elem = n_ktiles * 128 * free_len
    half = free_len // (4 if quarter else 2) \
        if free_len >= 1024 else free_len

    with ExitStack() as ctx:
        stage = ctx.enter_context(tc.tile_pool(name=f"{name}_stage", bufs=1))
        junkp = ctx.enter_context(tc.tile_pool(name=f"{name}_junk", bufs=1))

        wf_tiles = []
        for k in range(n_ktiles):
            wf = stage.tile([P, free_len], F32, tag=f"wf{k}")
            nc.sync.dma_start(out=wf, in_=wT_dram[k * P:(k + 1) * P, :])
            wf_tiles.append(wf)

        # pass 1: per-partition abs sums.  dve_heavy puts everything on DVE
        # (keeps ACT free when off the startup path); otherwise alternate
        # ACT/DVE by k so the pass wall-clock halves (startup-critical).
        asum = consts.tile([P, n_ktiles], F32, tag=f"{name}_asum")
        junk = junkp.tile([P, free_len], BF16, tag="junk")
        for k in range(n_ktiles):
            if dve_heavy or (k % 2 == 1):
                nc.vector.tensor_reduce(out=asum[:, k:k + 1],
                                        in_=wf_tiles[k],
                                        axis=mybir.AxisListType.X,
                                        op=ALU.add,
                                        apply_absolute_value=True)
            else:
                nc.scalar.activation(out=junk, in_=wf_tiles[k], func=AF.Abs,
                                     accum_out=asum[:, k:k + 1])
            if warm is not None and k % 4 == 1:
                warm()
        tot = consts.tile([P, 1], F32, tag=f"{name}_tot")
        nc.vector.tensor_reduce(out=tot, in_=asum, axis=mybir.AxisListType.X,
                                op=ALU.add)
        # broadcast-sum across partitions: ones128.T @ tot into the warm-up
        # PSUM bank (prologue-time; ring cycles are free)
        ones128 = junkp.tile([P, P], F32, tag="ones128")
        nc.vector.memset(ones128, 1.0)
        nc.tensor.matmul(out=warm_ps[:, 0:1], lhsT=ones128, rhs=tot,
                         start=True, stop=True)
        gsum = consts.tile([P, 1], F32, tag=f"{name}_gsum")
        nc.scalar.copy(out=gsum, in_=warm_ps[:, 0:1])
        # mean -> clip -> reciprocal scale
        meanclip = consts.tile([P, 1], F32, tag=f"{name}_meanclip")
        nc.vector.tensor_scalar(out=meanclip, in0=gsum, scalar1=1.0 / n_elem,
                                scalar2=1e-5, op0=ALU.mult, op1=ALU.max)
        swq = consts.tile([P, 1], F32, tag=f"{name}_swq")
        nc.vector.reciprocal(out=swq, in_=meanclip)

        # pass 2: round+clip to ternary bf16 in half-chunks (keeps the
        # rt/cl scratch tiles small)
        wq_tiles = []
        for k in range(n_ktiles):
            wq = consts.tile([P, free_len], BF16, tag=f"{name}_wq{k}")
            for h0 in range(0, free_len, half):
                hs = slice(h0, h0 + half)
                rt = junkp.tile([P, half], F32, tag="stage_rt", bufs=1)
                if dve_heavy or (k % 2 == 1):
                    nc.vector.tensor_scalar(out=rt, in0=wf_tiles[k][:, hs],
                                            scalar1=swq, scalar2=MAGIC,
                                            op0=ALU.mult, op1=ALU.add)
                else:
                    nc.scalar.activation(out=rt, in_=wf_tiles[k][:, hs],
                                         func=AF.Identity, bias=magicb,
                                         scale=swq)
                cl = junkp.tile([P, half], F32, tag="stage_cl", bufs=1)
                nc.vector.tensor_scalar(out=cl, in0=rt, scalar1=MAGIC,
                                        scalar2=1.0, op0=ALU.subtract,
                                        op1=ALU.min)
                nc.vector.tensor_scalar(out=wq[:, hs], in0=cl, scalar1=-1.0,
                                        scalar2=None, op0=ALU.max)
            wq_tiles.append(wq)
            if warm is not None and k % 4 == 3:
                warm()
    return wq_tiles, meanclip


def build_nc(general_g: bool):
    nc = bass.Bass()
    x_d = nc.dram_tensor("x", [TPC, H], F32, kind="ExternalInput")
    wupT_d = nc.dram_tensor("wupT", [H, I], F32, kind="ExternalInput")
    wdnT_d = nc.dram_tensor("wdnT", [I, H], F32, kind="ExternalInput")
    g_d = nc.dram_tensor("g", [I], F32, kind="ExternalInput")
    out_d = nc.dram_tensor("out", [TPC, H], F32, kind="ExternalOutput")

    from contextlib import ExitStack
    with ExitStack() as ctx:
        tc = ctx.enter_context(tile.TileContext(nc))

        # ---------------- constants / weight prep ----------------
        consts = ctx.enter_context(tc.tile_pool(name="consts", bufs=1))

        magicb = consts.tile([P, 1], F32)
        nc.vector.memset(magicb, MAGIC)
        nmagicb = consts.tile([P, 1], F32)
        nc.vector.memset(nmagicb, -MAGIC)

        g_bc = None
        if general_g:
            # g broadcast to all partitions: [128, I] f32
            g_bc = consts.tile([P, I], F32)
            g_ap = g_d[:]
            g_bcast_ap = bass.AP(tensor=g_ap.tensor, offset=g_ap.offset,
                                 ap=[[0, P]] + list(g_ap.ap))
            nc.gpsimd.dma_start(out=g_bc, in_=g_bcast_ap)

        # ---------------- PSUM pools (8 banks total) ----------------
        # ih: one contiguous 4-bank tile [P, 4, 512] f32; o: 4 banks.
        # The prologue (warm-up matmuls, weight-quant broadcast, g0
        # broadcast) borrows one o slot before the first down-matmul.
        ps_ih = ctx.enter_context(tc.tile_pool(name="ps_ih", bufs=1,
                                               space="PSUM"))
        ps_o = ctx.enter_context(tc.tile_pool(name="ps_o", bufs=4,
                                              space="PSUM"))

        # HAM warm-up: tiny dependency-free matmuls sprinkled through the
        # prologue emission keep the PE activity monitor at K=8/8 so the
        # first real matmuls run at 2.4 GHz instead of 1.2 GHz.
        warm_w = consts.tile([P, P], BF16)
        nc.vector.memset(warm_w, 1.0)
        warm_ps = ps_o.tile([P, H], F32, tag="o", name="warm_ps")

        def warm():
            nc.tensor.matmul(out=warm_ps[:, 0:P], lhsT=warm_w,
                             rhs=warm_w, start=True, stop=True)

        for _ in range(6):
            warm()

        g0b = consts.tile([P, 1], F32)
        with ExitStack() as gctx:
            gstage = gctx.enter_context(tc.tile_pool(name="gstage", bufs=1))
            # g0 broadcast [128,1] via K=1 matmul with ones
            ones_row = gstage.tile([1, P], F32, tag="ones_row")
            nc.vector.memset(ones_row, 1.0)
            g0_sb = gstage.tile([1, 1], F32, tag="g0sb")
            nc.gpsimd.dma_start(out=g0_sb, in_=g_d[0:1])
            nc.tensor.matmul(out=warm_ps[:, 0:1], lhsT=ones_row, rhs=g0_sb,
                             start=True, stop=True)
            nc.scalar.copy(out=g0b, in_=warm_ps[:, 0:1])

        # weights are quantized mid-prologue (below); placeholders for the
        # emit closures, assigned before first use.
        wup_q = wdn_q = None
        k1b = consts.tile([P, 1], F32)
        wdk = consts.tile([P, 1], F32)
        isg = consts.tile([P, 1], F32)
        g0a = consts.tile([P, 1], F32)

        def emit_gain_consts(up_meanclip, dn_meanclip):
            nc.vector.tensor_scalar_mul(out=k1b, in0=up_meanclip,
                                        scalar1=1.0 / 127.0)
            nc.scalar.activation(out=g0a, in_=g0b, func=AF.Abs)
            if general_g:
                # per-channel sign lives in sg/iu; wdk unsigned
                nc.vector.tensor_scalar_mul(out=wdk, in0=dn_meanclip,
                                            scalar1=1.0 / 127.0)
                nc.vector.memset(isg, 1.0 / 127.0)
            else:
                # dr is computed positive (Square path); fold sign(g0)
                # into the final output scale instead.
                sgn = consts.tile([P, 1], F32)
                nc.scalar.activation(out=sgn, in_=g0b, func=AF.Sign)
                wdk0 = consts.tile([P, 1], F32)
                nc.vector.tensor_scalar_mul(out=wdk0, in0=dn_meanclip,
                                            scalar1=1.0 / 127.0)
                nc.vector.tensor_tensor(out=wdk, in0=wdk0, in1=sgn,
                                        op=ALU.mult)

        # ---------------- main token-tile pipeline ----------------
        xs_bufs = 9 if general_g else 12
        xs_pool = ctx.enter_context(tc.tile_pool(name="xs", bufs=xs_bufs))
        xq_pool = ctx.enter_context(tc.tile_pool(name="xqp", bufs=1))
        ix_pool = ctx.enter_context(tc.tile_pool(name="ixp", bufs=2))
        xT_pool = ctx.enter_context(tc.tile_pool(name="xTp", bufs=5))
        rp = ctx.enter_context(tc.tile_pool(name="rp", bufs=2))
        ts1p = ctx.enter_context(tc.tile_pool(name="ts1p", bufs=2))
        sp = ctx.enter_context(tc.tile_pool(name="sp", bufs=2))
        sgp = (ctx.enter_context(tc.tile_pool(name="sgp", bufs=2))
               if general_g else None)
        iup = ctx.enter_context(tc.tile_pool(name="iup", bufs=2))
        iuTp = ctx.enter_context(tc.tile_pool(name="iuTp", bufs=4))
        junkp = ctx.enter_context(tc.tile_pool(name="mjunk", bufs=1))
        o2p = ctx.enter_context(tc.tile_pool(name="o2p", bufs=2))
        small = ctx.enter_context(tc.tile_pool(name="small", bufs=4))
        batchp = ctx.enter_context(tc.tile_pool(name="batchp", bufs=3))

        KV = 1.0 / I   # var = alpha^4 * sum(s^2) / I (q24 is raw both paths)

        abatch = {}        # ib8 -> x batch state (x_tiles, xm8, t08, xsc8)
        cbatch = {}        # ib4 -> beta batch state (Sm4, q24)
        tile_state = {}    # (key, t) -> live tile
        c_state = {}       # ib4 -> b4 output-scale tile

        def emit_Xdma(t):
            """One x-tile load per iteration (sync ring, emitted last)."""
            ib = (t // BGA) * BGA
            if t == ib:
                abatch[ib] = dict(x_tiles={}, xm8=batchp.tile(
                    [P, BGA], F32, tag="xm8", name="xm8"))
            x_sb = xs_pool.tile([P, H], F32, tag="x", bufs=xs_bufs)
            nc.sync.dma_start(out=x_sb, in_=x_d[t * P:(t + 1) * P, :])
            abatch[ib]["x_tiles"][t] = x_sb

        def emit_XM(t):
            """One per-token absmax per iteration."""
            ib = (t // BGA) * BGA
            bs = abatch[ib]
            j = t - ib
            nc.vector.tensor_reduce(out=bs["xm8"][:, j:j + 1],
                                    in_=bs["x_tiles"][t],
                                    axis=mybir.AxisListType.X, op=ALU.max,
                                    apply_absolute_value=True)

        def emit_XC(ib):
            """Batched x-scale chain for batch ib (after all 8 absmaxes)."""
            bs = abatch[ib]
            t08 = batchp.tile([P, BGA], F32, tag="t08")
            nc.vector.tensor_scalar_max(out=t08, in0=bs["xm8"], scalar1=1e-5)
            xr8 = batchp.tile([P, BGA], F32, tag="xr8")
            nc.vector.reciprocal(out=xr8, in_=t08)
            xsc8 = batchp.tile([P, BGA], F32, tag="xsc8")
            nc.vector.tensor_scalar_mul(out=xsc8, in0=xr8, scalar1=127.0)
            bs["t08"] = t08
            bs["xsc8"] = xsc8

        def emit_Q(t):
            """x-quant for tile t (xq on DVE 2x, ix on ACT) + xbar DMA
            transpose to xT."""
            ib = (t // BGA) * BGA
            bs = abatch[ib]
            j = t - ib
            x_sb = bs["x_tiles"].pop(t)
            xq = xq_pool.tile([P, H], F32, tag="xq")
            nc.vector.tensor_scalar(out=xq, in0=x_sb,
                                    scalar1=bs["xsc8"][:, j:j + 1],
                                    scalar2=MAGIC, op0=ALU.mult, op1=ALU.add)
            ix = ix_pool.tile([P, H], BF16, tag="ix")
            nc.scalar.activation(out=ix, in_=xq, func=AF.Identity,
                                 bias=nmagicb)
            xT = xT_pool.tile([P, NKH, P], BF16, tag="xT")
            nc.sync.dma_start_transpose(out=xT, in_=ix)
            tile_state[("xT", t)] = xT

        def emit_U(t):
            """Up matmul for tile t into the single 4-bank ih PSUM tile."""
            xT = tile_state.pop(("xT", t))
            ih = ps_ih.tile([P, NB, 512], F32, tag="ih")
            for q in range(NB):
                for k in range(NKH):
                    nc.tensor.matmul(
                        out=ih[:, q, :],
                        lhsT=xT[:, k, :],
                        rhs=wup_q[k][:, q * 512:(q + 1) * 512],
                        start=(k == 0), stop=(k == NKH - 1))
            tile_state[("ih", t)] = ih

        def emit_R(t):
            """relu drain of the whole 2048-wide ih in ONE ACT op."""
            ih = tile_state[("ih", t)]
            r_sb = rp.tile([P, I], F32, tag="r")
            nc.scalar.activation(out=r_sb,
                                 in_=ih.rearrange("p a b -> p (a b)"),
                                 func=AF.Relu)
            tile_state[("r", t)] = r_sb

        def emit_M(t):
            """Per-token stats straight off the PSUM ih, concurrent with
            the ACT relu drain (no relu->mr serialization): mr = max(ih),
            scc = max(max(mr,0)^2, 1e-30), dr = 127/scc.  general-g:
            nothing here (stats come off sg in emit_G)."""
            ib = (t // BGC) * BGC
            if t == ib:
                cbatch[ib] = dict(
                    Sm4=batchp.tile([P, BGC], F32, tag="Sm4", name="Sm4"),
                    q24=batchp.tile([P, BGC], F32, tag="q24", name="q24"))
            cs = cbatch[ib]
            ih = tile_state.pop(("ih", t))
            if general_g:
                return
            j = t - ib
            mr = small.tile([P, 1], F32, tag="mr")
            nc.vector.tensor_reduce(out=mr,
                                    in_=ih.rearrange("p a b -> p (a b)"),
                                    axis=mybir.AxisListType.X, op=ALU.max)
            m0 = small.tile([P, 1], F32, tag="m0")
            nc.vector.tensor_scalar_max(out=m0, in0=mr, scalar1=0.0)
            # scc = max(m0^2, 1e-30) (== max of f32-rounded squares)
            nc.vector.tensor_scalar(out=cs["Sm4"][:, j:j + 1], in0=m0,
                                    scalar1=m0, scalar2=1e-30,
                                    op0=ALU.mult, op1=ALU.max)
            sc2 = small.tile([P, 1], F32, tag="sc2")
            nc.vector.tensor_scalar_mul(out=sc2, in0=cs["Sm4"][:, j:j + 1],
                                        scalar1=1.0 / 127.0)
            dr = small.tile([P, 1], F32, tag="dr")
            nc.vector.reciprocal(out=dr, in_=sc2)      # dr = 127/scc > 0
            tile_state[("dr", t)] = dr

        def emit_s(t):
            """s = relu(ih)^2 on ACT (Square of the drained r)."""
            r_sb = tile_state.pop(("r", t))
            s_sb = sp.tile([P, I], F32, tag="s")
            nc.scalar.activation(out=s_sb, in_=r_sb, func=AF.Square)
            tile_state[("s", t)] = s_sb

        def emit_G(t):
            """general-g only: sg = s*g, Smax = max|sg|, dr = 127/clip."""
            ib = (t // BGC) * BGC
            j = t - ib
            cs = cbatch[ib]
            s_sb = tile_state[("s", t)]
            sg = sgp.tile([P, I], F32, tag="sg")
            nc.vector.tensor_tensor(out=sg, in0=s_sb, in1=g_bc, op=ALU.mult)
            nc.vector.tensor_reduce(out=cs["Sm4"][:, j:j + 1], in_=sg,
                                    axis=mybir.AxisListType.X, op=ALU.max,
                                    apply_absolute_value=True)
            sc2 = small.tile([P, 1], F32, tag="sc2")
            nc.vector.tensor_scalar(out=sc2, in0=cs["Sm4"][:, j:j + 1],
                                    scalar1=1e-30, scalar2=1.0 / 127.0,
                                    op0=ALU.max, op1=ALU.mult)
            dr = small.tile([P, 1], F32, tag="dr")
            nc.vector.reciprocal(out=dr, in_=sc2)
            tile_state[("sg", t)] = sg
            tile_state[("dr", t)] = dr

        def emit_I(t):
            """ts1 = s*dr + MAGIC (2x, f32) then iu = ts1 - MAGIC -> bf16
            (2x), + xbar DMA transpose to iuT."""
            dr = tile_state.pop(("dr", t))
            src = (tile_state.pop(("sg", t)) if general_g
                   else tile_state[("s", t)])
            ts1 = ts1p.tile([P, I], F32, tag="ts1")
            nc.vector.tensor_scalar(out=ts1, in0=src, scalar1=dr,
                                    scalar2=MAGIC, op0=ALU.mult, op1=ALU.add)
            iu = iup.tile([P, I], BF16, tag="iu")
            nc.vector.tensor_scalar(out=iu, in0=ts1, scalar1=MAGIC,
                                    scalar2=None, op0=ALU.subtract)
            iuT = iuTp.tile([P, NKI, P], BF16, tag="iuT")
            nc.sync.dma_start_transpose(out=iuT, in_=iu)
            tile_state[("iuT", t)] = iuT

        def emit_Q2(t):
            """q24 col = sum(s^2) on ACT (raw; KV = 1/I both paths)."""
            ib = (t // BGC) * BGC
            j = t - ib
            cs = cbatch[ib]
            junk2 = junkp.tile([P, I], BF16, tag="junk2")
            s_sb = tile_state.pop(("s", t))
            nc.scalar.activation(out=junk2, in_=s_sb, func=AF.Square,
                                 accum_out=cs["q24"][:, j:j + 1])

        def emit_D(t):
            """Down matmul for tile t (lhsT = DMA-transposed iuT blocks)."""
            iuT = tile_state.pop(("iuT", t))
            o_ps = ps_o.tile([P, H], F32, tag="o")
            for k in range(NKI):
                nc.tensor.matmul(out=o_ps,
                                 lhsT=iuT[:, k, :],
                                 rhs=wdn_q[k],
                                 start=(k == 0), stop=(k == NKI - 1))
            tile_state[("ops", t)] = o_ps

        def emit_C(ib):
            """Batched beta chain for tiles ib..ib+BGC-1."""
            cs = cbatch.pop(ib)
            iba = (ib // BGA) * BGA
            bs = abatch[iba]
            j4 = ib - iba
            t04 = bs["t08"][:, j4:j4 + BGC]
            Sm4, q24 = cs["Sm4"], cs["q24"]
            if general_g:
                scc4 = batchp.tile([P, BGC], F32, tag="scc4")
                nc.vector.tensor_scalar_max(out=scc4, in0=Sm4, scalar1=1e-30)
            else:
                scc4 = Sm4  # already max(max(mr,0)^2, 1e-30) from emit_M
            ga4 = batchp.tile([P, BGC], F32, tag="ga4")
            nc.vector.tensor_scalar_mul(out=ga4, in0=t04, scalar1=k1b)
            al4 = batchp.tile([P, BGC], F32, tag="al4")
            nc.vector.tensor_tensor(out=al4, in0=ga4, in1=ga4, op=ALU.mult)
            m14 = batchp.tile([P, BGC], F32, tag="m14")
            nc.vector.tensor_tensor(out=m14, in0=al4, in1=scc4, op=ALU.mult)
            al24 = batchp.tile([P, BGC], F32, tag="al24")
            nc.vector.tensor_tensor(out=al24, in0=al4, in1=al4, op=ALU.mult)
            v14 = batchp.tile([P, BGC], F32, tag="v14")
            nc.vector.tensor_tensor(out=v14, in0=al24, in1=q24, op=ALU.mult)
            Ve4 = batchp.tile([P, BGC], F32, tag="Ve4")
            nc.vector.tensor_scalar(out=Ve4, in0=v14, scalar1=KV,
                                    scalar2=EPS, op0=ALU.mult, op1=ALU.add)
            sq4 = batchp.tile([P, BGC], F32, tag="sq4")
            nc.scalar.activation(out=sq4, in_=Ve4, func=AF.Sqrt)
            cr4 = batchp.tile([P, BGC], F32, tag="cr4")
            nc.vector.reciprocal(out=cr4, in_=sq4)
            # one Newton step for rsqrt accuracy (ACT sqrt is approximate)
            h14 = batchp.tile([P, BGC], F32, tag="h14")
            nc.vector.tensor_tensor(out=h14, in0=cr4, in1=cr4, op=ALU.mult)
            h24 = batchp.tile([P, BGC], F32, tag="h24")
            nc.vector.tensor_tensor(out=h24, in0=h14, in1=Ve4, op=ALU.mult)
            h34 = batchp.tile([P, BGC], F32, tag="h34")
            nc.vector.tensor_scalar(out=h34, in0=h24, scalar1=-0.5,
                                    scalar2=1.5, op0=ALU.mult, op1=ALU.add)
            c4 = batchp.tile([P, BGC], F32, tag="c4")
            nc.vector.tensor_tensor(out=c4, in0=cr4, in1=h34, op=ALU.mult)
            if general_g:
                m1g4 = m14
            else:
                m1g4 = batchp.tile([P, BGC], F32, tag="m1g4")
                nc.vector.tensor_scalar_mul(out=m1g4, in0=m14, scalar1=g0a)
            mu4 = batchp.tile([P, BGC], F32, tag="mu4")
            nc.vector.tensor_tensor(out=mu4, in0=c4, in1=m1g4, op=ALU.mult)
            b4 = batchp.tile([P, BGC], F32, tag="b4")
            nc.vector.tensor_scalar(out=b4, in0=mu4, scalar1=1e-5,
                                    scalar2=wdk, op0=ALU.max, op1=ALU.mult)
            c_state[ib] = b4
            if ib % BGA == BGA - BGC:
                del abatch[iba]

        def emit_O(t):
            """Fused scale+drain of the down PSUM on DVE, then store."""
            ib = (t // BGC) * BGC
            j = t - ib
            b4 = c_state[ib]
            o_ps = tile_state.pop(("ops", t))
            o2 = o2p.tile([P, H], F32, tag="o2")
            nc.vector.tensor_scalar_mul(out=o2, in0=o_ps,
                                        scalar1=b4[:, j:j + 1])
            nc.sync.dma_start(out=out_d[t * P:(t + 1) * P, :], in_=o2)
            if j == BGC - 1:
                del c_state[ib]

        # ---- software-pipelined emission ----
        # Stage lags (iteration i):
        #   R(i) + M(i) concurrent off ih | s(i-1) | I(i-2) ts1/iu |
        #   Q2(i-2) | D(i-5) | C(i-6 batch) | O(i-7) | U(i+1) | Q(i+4) |
        #   XM(i+12) | XC(i+5 batch) | Xdma(i+13)
        # The only cross-engine hop inside a tile's chain is ih->{relu,mr};
        # everything else is DVE->DVE or ACT->ACT, so the strict-FIFO
        # engine queues never block on each other mid-iteration.
        wup_q, up_meanclip = _emit_weight_quant(
            nc, tc, consts, warm_ps, wupT_d, NKH, I, "wup", magicb,
            warm=warm, quarter=general_g)
        for t in range(13):
            emit_Xdma(t)
        for t in range(12):
            emit_XM(t)
        emit_XC(0)
        warm()
        emit_Q(0)
        emit_Q(1)
        warm()
        emit_Q(2)
        emit_Q(3)
        warm()
        emit_U(0)
        wdn_q, dn_meanclip = _emit_weight_quant(
            nc, tc, consts, warm_ps, wdnT_d, NKI, H, "wdn", magicb,
            dve_heavy=True, warm=None)
        emit_gain_consts(up_meanclip, dn_meanclip)
        for i in range(NT + 13):
            if 0 <= i < NT:
                emit_R(i)
            if 0 <= i - 1 < NT:
                emit_s(i - 1)
            if 0 <= i < NT:
                emit_M(i)
            if general_g and 0 <= i - 1 < NT:
                emit_G(i - 1)
            if (i - 6) % BGC == 0 and 0 <= i - 6 < NT:
                emit_C(i - 6)
            if 0 <= i - 7 < NT:
                emit_O(i - 7)
            if 0 <= i - 2 < NT:
                emit_I(i - 2)
            if 0 <= i - 2 < NT:
                emit_Q2(i - 2)
            if 0 <= i - 5 < NT:
                emit_D(i - 5)
            if 0 <= i + 1 < NT:
                emit_U(i + 1)
            if 0 <= i + 4 < NT:
                emit_Q(i + 4)
            if 0 <= i + 12 < NT:
                emit_XM(i + 12)
            if (i + 5) % BGA == 0 and 0 <= i + 5 < NT:
                emit_XC(i + 5)
            if 0 <= i + 13 < NT:
                emit_Xdma(i + 13)

    _split_sync_waits(nc)
    return nc


_NC_CACHE = {}


def kernel(x, w_up, w_down, g):
    global LAST_RESULT
    x = np.ascontiguousarray(x, dtype=np.float32)
    w_up = np.ascontiguousarray(w_up, dtype=np.float32)
    w_down = np.ascontiguousarray(w_down, dtype=np.float32)
    g = np.ascontiguousarray(g, dtype=np.float32)

    if abs(float(g[0])) < 1e-30 and np.all(g == g[0]):
        return np.zeros_like(x)

    general = not bool(np.all(g == g[0]))
    key = ("gen" if general else "const")
    if key not in _NC_CACHE:
        _NC_CACHE[key] = build_nc(general)
    nc = _NC_CACHE[key]

    xt = x.reshape(TOK, H)
    wupT = np.ascontiguousarray(w_up.T)    # [H, I]
    wdnT = np.ascontiguousarray(w_down.T)  # [I, H]
    in_maps = [
        {"x": xt[c * TPC:(c + 1) * TPC], "wupT": wupT, "wdnT": wdnT, "g": g}
        for c in range(N_CORES)
    ]
    res = run_bass_kernel_spmd(
        nc, in_maps, list(range(N_CORES)),
        trace=bool(os.environ.get("BASS_TRACE")),
    )
    LAST_RESULT = res
    out = np.concatenate([res.results[c]["out"] for c in range(N_CORES)],
                         axis=0)
    return out.reshape(B, S, H)
